# revision 3
# baseline (speedup 1.0000x reference)
"""AdaptiveRouter (MoE routing) Trainium2 kernel — 8-core data-parallel.

Strategy:
  - Shard token dim N=32768 across 8 cores (4096 tokens each).
  - Host-side input prep (layout only, no math beyond dtype split):
      * each core's x shard is transposed to [D=1024, 4096] and split into
        bf16 hi/lo halves so the gate matmul runs as 3 bf16 matmuls
        (x_hi@W_hi + x_hi@W_lo + x_lo@W_hi) with near-fp32 accuracy at
        3x bf16 TensorE throughput (measured max err ~5e-6).
      * W_gate likewise hi/lo split; TTHA weights stay fp32.
  - The tiny batch-1 TTHA adapter is computed on every core (replicated;
    cheaper than a collective) in fp32, overlapped with the x DMA stream.
  - Per 512-token group: 24 bf16 matmuls -> PSUM [64experts, 512tok],
    fused (b_gate + routing_bias) add, PE-transpose to [128tok, 64], then
    DVE Max8/MaxIndex for top-2, batched top-2 softmax epilogue.
  - No collectives: outputs are gathered host-side.
"""

import sys

sys.path.insert(0, "/opt/trn_rl_repo")

import numpy as np
import ml_dtypes

import concourse.bass as bass
import concourse.mybir as mybir
import concourse.tile as tile
from concourse import bacc
from concourse.bass_utils import run_bass_kernel_spmd
from concourse.masks import make_identity

F32 = mybir.dt.float32
BF16 = mybir.dt.bfloat16
I32 = mybir.dt.int32
U32 = mybir.dt.uint32
AF = mybir.ActivationFunctionType
OP = mybir.AluOpType
AX = mybir.AxisListType

N_CORES = 8
N, D, E, K = 32768, 1024, 64, 2
NT = N // N_CORES            # 4096 tokens per core
H = 256
G_TOK = 512                  # tokens per matmul group
N_GROUPS = NT // G_TOK       # 8
TILES = NT // 128            # 32 token tiles of 128

BF = ml_dtypes.bfloat16


def _act(nc, out, in_, func, scale=1.0):
    nc.scalar.activation(out, in_, func, scale=scale)


def _layer_norm_gelu(nc, sb, x_ap, g_ap, b_ap, out_ap, groups, width, tag):
    """out = gelu(LN(x) * g + b) on a single partition.

    x_ap/out_ap: [1, groups*width] viewed as [1, groups, width];
    g_ap/b_ap: [1, groups*width]. LN per group of `width`.
    """
    x3 = x_ap.rearrange("o (g w) -> o g w", g=groups)
    g3 = g_ap.rearrange("o (g w) -> o g w", g=groups)
    b3 = b_ap.rearrange("o (g w) -> o g w", g=groups)
    o3 = out_ap.rearrange("o (g w) -> o g w", g=groups)

    mu = sb.tile([1, groups], F32, tag=f"{tag}_mu")
    nc.vector.tensor_reduce(mu[:], x3, axis=AX.X, op=OP.add)
    nc.vector.tensor_scalar(mu[:], mu[:], 1.0 / width, None, op0=OP.mult)

    xc = sb.tile([1, groups, width], F32, tag=f"{tag}_xc")
    nc.vector.tensor_tensor(xc[:], x3, mu[:, :, None].to_broadcast([1, groups, width]),
                            op=OP.subtract)

    sq = sb.tile([1, groups, width], F32, tag=f"{tag}_sq")
    nc.vector.tensor_tensor(sq[:], xc[:], xc[:], op=OP.mult)
    var = sb.tile([1, groups], F32, tag=f"{tag}_var")
    nc.vector.tensor_reduce(var[:], sq[:], axis=AX.X, op=OP.add)
    # var/width + eps
    nc.vector.tensor_scalar(var[:], var[:], 1.0 / width, 1e-5, op0=OP.mult, op1=OP.add)
    # inv_std = exp(-0.5 * ln(var+eps))
    _act(nc, var[:], var[:], AF.Ln)
    _act(nc, var[:], var[:], AF.Exp, scale=-0.5)

    nc.vector.tensor_tensor(xc[:], xc[:], var[:, :, None].to_broadcast([1, groups, width]),
                            op=OP.mult)
    nc.vector.tensor_tensor(xc[:], xc[:], g3, op=OP.mult)
    nc.vector.tensor_tensor(xc[:], xc[:], b3, op=OP.add)
    _act(nc, o3, xc[:], AF.Gelu)


def build():
    nc = bacc.Bacc(target_bir_lowering=False)

    # ---- DRAM parameters (per-core shards / replicated weights) ----
    xhi = nc.dram_tensor("xhi", [D, NT], BF16, kind="ExternalInput")
    xlo = nc.dram_tensor("xlo", [D, NT], BF16, kind="ExternalInput")
    wghi = nc.dram_tensor("wghi", [D, E], BF16, kind="ExternalInput")
    wglo = nc.dram_tensor("wglo", [D, E], BF16, kind="ExternalInput")
    bgate = nc.dram_tensor("bgate", [1, E], F32, kind="ExternalInput")

    cat_in = nc.dram_tensor("cat_in", [512, 1], F32, kind="ExternalInput")
    w_cat = nc.dram_tensor("w_cat", [512, 2 * H], F32, kind="ExternalInput")
    emb_bias = nc.dram_tensor("emb_bias", [1, 2 * H], F32, kind="ExternalInput")
    ln1_g = nc.dram_tensor("ln1_g", [1, 2 * H], F32, kind="ExternalInput")
    ln1_b = nc.dram_tensor("ln1_b", [1, 2 * H], F32, kind="ExternalInput")

    wqkv = nc.dram_tensor("wqkv", [H, 3 * H], F32, kind="ExternalInput")
    bqkv3 = nc.dram_tensor("bqkv3", [3, 3 * H], F32, kind="ExternalInput")
    wo = nc.dram_tensor("wo", [H, H], F32, kind="ExternalInput")
    bo3 = nc.dram_tensor("bo3", [3, H], F32, kind="ExternalInput")

    wf = nc.dram_tensor("wf", [H, H], F32, kind="ExternalInput")
    bf1 = nc.dram_tensor("bf1", [1, H], F32, kind="ExternalInput")
    gf1 = nc.dram_tensor("gf1", [1, H], F32, kind="ExternalInput")
    bef1 = nc.dram_tensor("bef1", [1, H], F32, kind="ExternalInput")

    wo1 = nc.dram_tensor("wo1", [H, H // 2], F32, kind="ExternalInput")
    bo1 = nc.dram_tensor("bo1", [1, H // 2], F32, kind="ExternalInput")
    wo2 = nc.dram_tensor("wo2", [H // 2, E], F32, kind="ExternalInput")
    bo2 = nc.dram_tensor("bo2", [1, E], F32, kind="ExternalInput")
    wu1 = nc.dram_tensor("wu1", [H, H // 4], F32, kind="ExternalInput")
    bu1 = nc.dram_tensor("bu1", [1, H // 4], F32, kind="ExternalInput")
    wu2 = nc.dram_tensor("wu2", [H // 4, E], F32, kind="ExternalInput")
    bu2 = nc.dram_tensor("bu2", [1, E], F32, kind="ExternalInput")

    out_w = nc.dram_tensor("out_w", [128, TILES * 2], F32, kind="ExternalOutput")
    out_i = nc.dram_tensor("out_i", [128, TILES * 2], I32, kind="ExternalOutput")
    out_rb = nc.dram_tensor("out_rb", [1, E], F32, kind="ExternalOutput")
    out_un = nc.dram_tensor("out_un", [1, E], F32, kind="ExternalOutput")

    with tile.TileContext(nc) as tc:
        with tc.tile_pool(name="const", bufs=1) as cs, \
             tc.tile_pool(name="tt", bufs=1) as ts, \
             tc.tile_pool(name="xs", bufs=3) as xs, \
             tc.tile_pool(name="wk", bufs=3) as wk, \
             tc.tile_pool(name="pmain", bufs=3, space="PSUM") as pmain, \
             tc.tile_pool(name="ptr", bufs=2, space="PSUM") as ptr, \
             tc.tile_pool(name="ptt", bufs=3, space="PSUM") as ptt:

            ident = cs.tile([128, 128], F32)
            make_identity(nc, ident[:])

            # ---- persistent result buffers ----
            vbuf8 = cs.tile([128, TILES * 8], F32)
            ibuf8 = cs.tile([128, TILES * 8], U32)
            wbuf = cs.tile([128, TILES, 2], F32)
            obuf = cs.tile([128, TILES, 2], I32)

            # ---- load gate weights ----
            whi_s = cs.tile([128, D // 128, E], BF16)
            wlo_s = cs.tile([128, D // 128, E], BF16)
            nc.sync.dma_start(whi_s[:], wghi[:].rearrange("(c p) e -> p c e", p=128))
            nc.sync.dma_start(wlo_s[:], wglo[:].rearrange("(c p) e -> p c e", p=128))

            # =========================================================
            # TTHA adapter (batch-1, replicated on every core, fp32)
            # =========================================================
            catT = ts.tile([128, 4, 1], F32)
            nc.sync.dma_start(catT[:], cat_in[:].rearrange("(c p) o -> p c o", p=128))
            wcat_s = ts.tile([128, 4, 2 * H], F32)
            nc.sync.dma_start(wcat_s[:], w_cat[:].rearrange("(c p) n -> p c n", p=128))
            embb_s = ts.tile([1, 2 * H], F32)
            nc.sync.dma_start(embb_s[:], emb_bias[:])
            ln1g_s = ts.tile([1, 2 * H], F32)
            nc.sync.dma_start(ln1g_s[:], ln1_g[:])
            ln1b_s = ts.tile([1, 2 * H], F32)
            nc.sync.dma_start(ln1b_s[:], ln1_b[:])

            ps_emb = ptt.tile([1, 2 * H], F32, tag="ptt")
            for c in range(4):
                nc.tensor.matmul(ps_emb[:], catT[:, c, :], wcat_s[:, c, :],
                                 start=(c == 0), stop=(c == 3))
            e0 = ts.tile([1, 2 * H], F32)
            nc.vector.tensor_tensor(e0[:], ps_emb[:], embb_s[:], op=OP.add)
            emb_act = ts.tile([1, 2 * H], F32)
            _layer_norm_gelu(nc, ts, e0[:], ln1g_s[:], ln1b_s[:], emb_act[:],
                             groups=2, width=H, tag="ln1")

            # combT [128, 2, 3]: cols (cost, hw, zero), chunks over H=256
            combT = ts.tile([128, 2, 3], F32)
            nc.vector.memset(combT[:], 0.0)
            for c in range(2):
                ps_cb = ptt.tile([128, 2], F32, tag="ptt")
                for j in range(2):
                    nc.tensor.transpose(ps_cb[:, j:j + 1],
                                        emb_act[0:1, (j * H + c * 128):(j * H + c * 128 + 128)],
                                        ident[0:1, 0:1])
                nc.vector.tensor_copy(combT[:, c, 0:2], ps_cb[:])

            # qkv = comb @ Wqkv + bqkv  -> [3, 768]
            wqkv_s = ts.tile([128, 2, 3 * H], F32)
            nc.sync.dma_start(wqkv_s[:], wqkv[:].rearrange("(c p) n -> p c n", p=128))
            bqkv_s = ts.tile([3, 3 * H], F32)
            nc.sync.dma_start(bqkv_s[:], bqkv3[:])
            qkv_s = ts.tile([3, 3 * H], F32)
            for half in range(2):
                ps_q = ptt.tile([3, 384], F32, tag="ptt")
                for c in range(2):
                    nc.tensor.matmul(ps_q[:], combT[:, c, :],
                                     wqkv_s[:, c, (half * 384):(half * 384 + 384)],
                                     start=(c == 0), stop=(c == 1))
                nc.vector.tensor_tensor(qkv_s[:, half * 384:(half + 1) * 384], ps_q[:],
                                        bqkv_s[:, half * 384:(half + 1) * 384], op=OP.add)

            # qT (scaled by 1/sqrt(32)) and kT packed 3 heads per chunk so
            # matmul operand base partitions stay in {0, 32, 64}.
            qT = ts.tile([128, 3, 3], F32)
            kT = ts.tile([128, 3, 3], F32)
            scale = 1.0 / np.sqrt(32.0)
            for c in range(3):
                w = 96 if c < 2 else 64
                ps_t = ptt.tile([128, 3], F32, tag="ptt")
                nc.tensor.transpose(ps_t[:w], qkv_s[0:3, c * 96:(c * 96 + w)],
                                    ident[0:3, 0:3])
                nc.vector.tensor_scalar(qT[:w, c, :], ps_t[:w], scale, None, op0=OP.mult)
                ps_t2 = ptt.tile([128, 3], F32, tag="ptt")
                nc.tensor.transpose(ps_t2[:w], qkv_s[0:3, (H + c * 96):(H + c * 96 + w)],
                                    ident[0:3, 0:3])
                nc.vector.tensor_copy(kT[:w, c, :], ps_t2[:w])

            # scores per head -> [3, 8, 3]
            ps_sc = ptt.tile([3, 24], F32, tag="ptt")
            for h in range(8):
                c, hh = divmod(h, 3)
                nc.tensor.matmul(ps_sc[:, h * 3:(h + 1) * 3],
                                 qT[hh * 32:(hh + 1) * 32, c, :],
                                 kT[hh * 32:(hh + 1) * 32, c, :],
                                 start=True, stop=True)
            s_sc = ts.tile([3, 8, 3], F32)
            nc.vector.tensor_copy(s_sc[:], ps_sc[:].rearrange("p (h j) -> p h j", h=8))
            rmax = ts.tile([3, 8], F32)
            nc.vector.tensor_reduce(rmax[:], s_sc[:], axis=AX.X, op=OP.max)
            nc.vector.tensor_tensor(s_sc[:], s_sc[:],
                                    rmax[:, :, None].to_broadcast([3, 8, 3]), op=OP.subtract)
            _act(nc, s_sc[:], s_sc[:], AF.Exp)
            rsum = ts.tile([3, 8], F32)
            nc.vector.tensor_reduce(rsum[:], s_sc[:], axis=AX.X, op=OP.add)
            nc.vector.reciprocal(rsum[:], rsum[:])
            nc.vector.tensor_tensor(s_sc[:], s_sc[:],
                                    rsum[:, :, None].to_broadcast([3, 8, 3]), op=OP.mult)

            # attnT per head, ctx = attn @ v -> [3, 256]
            at_s = ts.tile([3, 8, 3], F32)
            ps_at = ptt.tile([3, 24], F32, tag="ptt")
            for h in range(8):
                nc.tensor.transpose(ps_at[:, h * 3:(h + 1) * 3], s_sc[:, h, :],
                                    ident[0:3, 0:3])
            nc.vector.tensor_copy(at_s[:], ps_at[:].rearrange("p (h j) -> p h j", h=8))
            ps_ctx = ptt.tile([3, H], F32, tag="ptt")
            for h in range(8):
                nc.tensor.matmul(ps_ctx[:, h * 32:(h + 1) * 32], at_s[:, h, :],
                                 qkv_s[0:3, (2 * H + h * 32):(2 * H + (h + 1) * 32)],
                                 start=True, stop=True)
            ctx_s = ts.tile([3, H], F32)
            nc.vector.tensor_copy(ctx_s[:], ps_ctx[:])

            # attended = ctx @ Wo + bo -> [3, 256]
            ctxT = ts.tile([128, 2, 3], F32)
            for c in range(2):
                ps_ct = ptt.tile([128, 3], F32, tag="ptt")
                nc.tensor.transpose(ps_ct[:], ctx_s[0:3, c * 128:(c + 1) * 128],
                                    ident[0:3, 0:3])
                nc.vector.tensor_copy(ctxT[:, c, :], ps_ct[:])
            wo_s = ts.tile([128, 2, H], F32)
            nc.sync.dma_start(wo_s[:], wo[:].rearrange("(c p) n -> p c n", p=128))
            bo_s = ts.tile([3, H], F32)
            nc.sync.dma_start(bo_s[:], bo3[:])
            ps_att = ptt.tile([3, H], F32, tag="ptt")
            for c in range(2):
                nc.tensor.matmul(ps_att[:], ctxT[:, c, :], wo_s[:, c, :],
                                 start=(c == 0), stop=(c == 1))
            att_s = ts.tile([3, H], F32)
            nc.vector.tensor_tensor(att_s[:], ps_att[:], bo_s[:], op=OP.add)

            # mean over the 3 rows -> [1, 256]
            third = ts.tile([3, 1], F32)
            nc.vector.memset(third[:], 1.0 / 3.0)
            ps_mean = ptt.tile([1, H], F32, tag="ptt")
            nc.tensor.matmul(ps_mean[:], third[:], att_s[:], start=True, stop=True)
            mean_s = ts.tile([1, H], F32)
            nc.vector.tensor_copy(mean_s[:], ps_mean[:])

            # fused = gelu(LN(mean @ Wf + bf)) -> [1, 256]
            meanT = ts.tile([128, 2, 1], F32)
            for c in range(2):
                ps_mt = ptt.tile([128, 1], F32, tag="ptt")
                nc.tensor.transpose(ps_mt[:], mean_s[0:1, c * 128:(c + 1) * 128],
                                    ident[0:1, 0:1])
                nc.vector.tensor_copy(meanT[:, c, :], ps_mt[:])
            wf_s = ts.tile([128, 2, H], F32)
            nc.sync.dma_start(wf_s[:], wf[:].rearrange("(c p) n -> p c n", p=128))
            ps_f = ptt.tile([1, H], F32, tag="ptt")
            for c in range(2):
                nc.tensor.matmul(ps_f[:], meanT[:, c, :], wf_s[:, c, :],
                                 start=(c == 0), stop=(c == 1))
            bf_s = ts.tile([1, H], F32)
            nc.sync.dma_start(bf_s[:], bf1[:])
            gf_s = ts.tile([1, H], F32)
            nc.sync.dma_start(gf_s[:], gf1[:])
            bef_s = ts.tile([1, H], F32)
            nc.sync.dma_start(bef_s[:], bef1[:])
            f0 = ts.tile([1, H], F32)
            nc.vector.tensor_tensor(f0[:], ps_f[:], bf_s[:], op=OP.add)
            fused = ts.tile([1, H], F32)
            _layer_norm_gelu(nc, ts, f0[:], gf_s[:], bef_s[:], fused[:],
                             groups=1, width=H, tag="ln2")

            fusedT = ts.tile([128, 2, 1], F32)
            for c in range(2):
                ps_ft = ptt.tile([128, 1], F32, tag="ptt")
                nc.tensor.transpose(ps_ft[:], fused[0:1, c * 128:(c + 1) * 128],
                                    ident[0:1, 0:1])
                nc.vector.tensor_copy(fusedT[:, c, :], ps_ft[:])

            # ---- routing bias head: tanh(gelu(fused@Wo1+bo1)@Wo2+bo2) ----
            wo1_s = ts.tile([128, 2, H // 2], F32)
            nc.sync.dma_start(wo1_s[:], wo1[:].rearrange("(c p) n -> p c n", p=128))
            ps_h1 = ptt.tile([1, H // 2], F32, tag="ptt")
            for c in range(2):
                nc.tensor.matmul(ps_h1[:], fusedT[:, c, :], wo1_s[:, c, :],
                                 start=(c == 0), stop=(c == 1))
            bo1_s = ts.tile([1, H // 2], F32)
            nc.sync.dma_start(bo1_s[:], bo1[:])
            h1 = ts.tile([1, H // 2], F32)
            nc.vector.tensor_tensor(h1[:], ps_h1[:], bo1_s[:], op=OP.add)
            _act(nc, h1[:], h1[:], AF.Gelu)
            h1T = ts.tile([128, 1], F32)
            ps_h1t = ptt.tile([128, 1], F32, tag="ptt")
            nc.tensor.transpose(ps_h1t[:], h1[0:1, :], ident[0:1, 0:1])
            nc.vector.tensor_copy(h1T[:], ps_h1t[:])
            wo2_s = ts.tile([128, E], F32)
            nc.sync.dma_start(wo2_s[:], wo2[:])
            ps_rb = ptt.tile([1, E], F32, tag="ptt")
            nc.tensor.matmul(ps_rb[:], h1T[:], wo2_s[:], start=True, stop=True)
            bo2_s = ts.tile([1, E], F32)
            nc.sync.dma_start(bo2_s[:], bo2[:])
            rb = ts.tile([1, E], F32)
            nc.vector.tensor_tensor(rb[:], ps_rb[:], bo2_s[:], op=OP.add)
            _act(nc, rb[:], rb[:], AF.Tanh)
            nc.sync.dma_start(out_rb[:], rb[:])

            # ---- uncertainty head: softplus(gelu(fused@Wu1+bu1)@Wu2+bu2) ----
            wu1_s = ts.tile([128, 2, H // 4], F32)
            nc.sync.dma_start(wu1_s[:], wu1[:].rearrange("(c p) n -> p c n", p=128))
            ps_u1 = ptt.tile([1, H // 4], F32, tag="ptt")
            for c in range(2):
                nc.tensor.matmul(ps_u1[:], fusedT[:, c, :], wu1_s[:, c, :],
                                 start=(c == 0), stop=(c == 1))
            bu1_s = ts.tile([1, H // 4], F32)
            nc.sync.dma_start(bu1_s[:], bu1[:])
            u1 = ts.tile([1, H // 4], F32)
            nc.vector.tensor_tensor(u1[:], ps_u1[:], bu1_s[:], op=OP.add)
            _act(nc, u1[:], u1[:], AF.Gelu)
            u1T = ts.tile([64, 1], F32)
            ps_u1t = ptt.tile([64, 1], F32, tag="ptt")
            nc.tensor.transpose(ps_u1t[:], u1[0:1, :], ident[0:1, 0:1])
            nc.vector.tensor_copy(u1T[:], ps_u1t[:])
            wu2_s = ts.tile([64, E], F32)
            nc.sync.dma_start(wu2_s[:], wu2[:])
            ps_u2 = ptt.tile([1, E], F32, tag="ptt")
            nc.tensor.matmul(ps_u2[:], u1T[:], wu2_s[:], start=True, stop=True)
            bu2_s = ts.tile([1, E], F32)
            nc.sync.dma_start(bu2_s[:], bu2[:])
            un = ts.tile([1, E], F32)
            nc.vector.tensor_tensor(un[:], ps_u2[:], bu2_s[:], op=OP.add)
            # softplus = ln(1 + exp(x))
            _act(nc, un[:], un[:], AF.Exp)
            nc.vector.tensor_scalar(un[:], un[:], 1.0, None, op0=OP.add)
            _act(nc, un[:], un[:], AF.Ln)
            nc.sync.dma_start(out_un[:], un[:])

            # ---- total per-expert bias as per-partition scalar [64, 1] ----
            bg_s = ts.tile([1, E], F32)
            nc.sync.dma_start(bg_s[:], bgate[:])
            bt = ts.tile([1, E], F32)
            nc.vector.tensor_tensor(bt[:], rb[:], bg_s[:], op=OP.add)
            ps_bt = ptt.tile([64, 1], F32, tag="ptt")
            nc.tensor.transpose(ps_bt[:], bt[0:1, :], ident[0:1, 0:1])
            bias_T = ts.tile([64, 1], F32)
            nc.vector.tensor_copy(bias_T[:], ps_bt[:])

            # =========================================================
            # Main gate matmul + top-2 loop
            # =========================================================
            for g in range(N_GROUPS):
                xhi_g = xs.tile([128, D // 128, G_TOK], BF16, tag="xhi")
                xlo_g = xs.tile([128, D // 128, G_TOK], BF16, tag="xlo")
                src_hi = xhi[:, g * G_TOK:(g + 1) * G_TOK].rearrange(
                    "(c p) t -> p c t", p=128)
                src_lo = xlo[:, g * G_TOK:(g + 1) * G_TOK].rearrange(
                    "(c p) t -> p c t", p=128)
                nc.sync.dma_start(xhi_g[:], src_hi)
                nc.sync.dma_start(xlo_g[:], src_lo)

                ps_lg = pmain.tile([E, G_TOK], F32, tag="lg")
                n_mm = 3 * (D // 128)
                k = 0
                for c in range(D // 128):
                    for (w_t, x_t) in ((whi_s, xhi_g), (whi_s, xlo_g), (wlo_s, xhi_g)):
                        nc.tensor.matmul(ps_lg[:], w_t[:, c, :], x_t[:, c, :],
                                         start=(k == 0), stop=(k == n_mm - 1))
                        k += 1

                adj = wk.tile([E, G_TOK], F32, tag="adj")
                nc.vector.tensor_scalar(adj[:], ps_lg[:], bias_T[:], None, op0=OP.add)

                for t in range(4):
                    tl = g * 4 + t
                    ps_tr = ptr.tile([128, E], F32, tag="tr")
                    nc.tensor.transpose(ps_tr[:], adj[:, t * 128:(t + 1) * 128],
                                        ident[0:E, 0:E])
                    adjT = wk.tile([128, E], F32, tag="adjT")
                    nc.vector.tensor_copy(adjT[:], ps_tr[:])
                    nc.vector.max(vbuf8[:, tl * 8:(tl + 1) * 8], adjT[:])
                    nc.vector.max_index(ibuf8[:, tl * 8:(tl + 1) * 8],
                                        vbuf8[:, tl * 8:(tl + 1) * 8], adjT[:])

            # ---- top-2 softmax epilogue (batched) ----
            v3 = vbuf8[:].rearrange("p (t k) -> p t k", k=8)
            i3 = ibuf8[:].rearrange("p (t k) -> p t k", k=8)
            d_t = wk.tile([128, TILES, 1], F32, tag="dt")
            nc.vector.tensor_tensor(d_t[:], v3[:, :, 1:2], v3[:, :, 0:1], op=OP.subtract)
            _act(nc, d_t[:], d_t[:], AF.Exp)
            s_t = wk.tile([128, TILES, 1], F32, tag="st")
            nc.vector.tensor_scalar(s_t[:], d_t[:], 1.0, None, op0=OP.add)
            nc.vector.reciprocal(wbuf[:, :, 0:1], s_t[:])
            nc.vector.tensor_tensor(wbuf[:, :, 1:2], d_t[:], wbuf[:, :, 0:1], op=OP.mult)
            nc.vector.tensor_copy(obuf[:, :, 0:1], i3[:, :, 0:1])
            nc.vector.tensor_copy(obuf[:, :, 1:2], i3[:, :, 1:2])
            nc.sync.dma_start(out_w[:], wbuf[:])
            nc.sync.dma_start(out_i[:], obuf[:])

    nc.finalize()
    return nc


_CACHE = {}


def _get_nc():
    if "nc" not in _CACHE:
        _CACHE["nc"] = build()
    return _CACHE["nc"]


def kernel(**inputs):
    f32 = np.float32
    g = {k: np.asarray(v, f32) for k, v in inputs.items()}
    x = g["x"]

    wg = g["W_gate"]
    wghi = wg.astype(BF)
    wglo = (wg - wghi.astype(f32)).astype(BF)

    cat_in = np.zeros((512, 1), f32)
    cat_in[:384, 0] = g["cost_features"][0]
    cat_in[384:392, 0] = g["hardware_features"][0]
    w_cat = np.zeros((512, 2 * H), f32)
    w_cat[:384, :H] = g["Wc"]
    w_cat[384:392, H:] = g["Wh"]

    shared = dict(
        wghi=wghi, wglo=wglo,
        bgate=g["b_gate"].reshape(1, E),
        cat_in=cat_in, w_cat=w_cat,
        emb_bias=np.concatenate([g["bc"], g["bh"]]).reshape(1, 2 * H),
        ln1_g=np.concatenate([g["gc"], g["gh"]]).reshape(1, 2 * H),
        ln1_b=np.concatenate([g["bec"], g["beh"]]).reshape(1, 2 * H),
        wqkv=g["Wqkv"], bqkv3=np.tile(g["bqkv"].reshape(1, 3 * H), (3, 1)),
        wo=g["Wo"], bo3=np.tile(g["bo"].reshape(1, H), (3, 1)),
        wf=g["Wf"], bf1=g["bf"].reshape(1, H),
        gf1=g["gf"].reshape(1, H), bef1=g["bef"].reshape(1, H),
        wo1=g["Wo1"], bo1=g["bo1"].reshape(1, H // 2),
        wo2=g["Wo2"], bo2=g["bo2"].reshape(1, E),
        wu1=g["Wu1"], bu1=g["bu1"].reshape(1, H // 4),
        wu2=g["Wu2"], bu2=g["bu2"].reshape(1, E),
    )
    shared = {k: np.ascontiguousarray(v, f32) if v.dtype != BF else v
              for k, v in shared.items()}

    in_maps = []
    for c in range(N_CORES):
        xs = np.ascontiguousarray(x[c * NT:(c + 1) * NT].T)
        xhi = xs.astype(BF)
        xlo = (xs - xhi.astype(f32)).astype(BF)
        in_maps.append(dict(shared, xhi=xhi, xlo=xlo))

    nc = _get_nc()
    res = run_bass_kernel_spmd(nc, in_maps, core_ids=list(range(N_CORES)))

    weights = np.empty((N, K), f32)
    top_idx = np.empty((N, K), np.int32)
    for c in range(N_CORES):
        r = res.results[c]
        weights[c * NT:(c + 1) * NT] = (
            r["out_w"].reshape(128, TILES, 2).transpose(1, 0, 2).reshape(NT, 2))
        top_idx[c * NT:(c + 1) * NT] = (
            r["out_i"].reshape(128, TILES, 2).transpose(1, 0, 2).reshape(NT, 2))
    rb = res.results[0]["out_rb"].reshape(1, E).astype(f32)
    un = res.results[0]["out_un"].reshape(1, E).astype(f32)
    return weights, top_idx, rb, un
